# revision 10
# baseline (speedup 1.0000x reference)
"""Trainium2 Bass kernel for nn_DecoupledMoEContainer_52596169507001.

Strategy: data-parallel over batch, 2 samples per core across 8 NeuronCores.
Host-side routing: per-sample expert weights are gathered by `indices` so the
device program is uniform (the NEFF does not depend on input values).

Per sample on device:
  shared: s = conv3x3(x)            -> 9 accumulated fp32r matmuls per row-tile
          BN (batch stats)          -> per-channel sum/sumsq, cross-core
                                       AllReduce of [128,2], then SiLU apply
  expert: h1 = conv1x1 (C->HID)     -> 2 fp32r matmuls per row-tile
          GN(8)+SiLU                -> bn_stats + tiny mask-matmuls + rsqrt
          h2 = conv1x1 (HID->C)     -> 2 accumulated matmuls per row-tile
          GN(8)                     -> folded into final combine
  out = silu(bn(s)) + pass*x + (h2 - m2')*cs      (routing coefs from host)
"""

import os

import numpy as np

import concourse.bacc as bacc
import concourse.bass as bass
import concourse.mybir as mybir
import concourse.tile as tile

F32 = mybir.dt.float32
F32R = mybir.dt.float32r
BF16 = mybir.dt.bfloat16
I32 = mybir.dt.int32
AF = mybir.ActivationFunctionType
ALU = mybir.AluOpType

# Problem shapes (hardcoded per spec)
B, C, HID, H, W = 16, 128, 256, 80, 80
NPIX = H * W                     # 6400
NCORES = 8
SPC = B // NCORES                # samples per core = 2
EPS = 1e-5

ROWS_PER_TILE = 5
NTILES = H // ROWS_PER_TILE      # 16
TILE_N = ROWS_PER_TILE * W       # 400
CHUNK = 800
NCHUNKS = NPIX // CHUNK          # 8
PADW = W + 2                     # 82
PADLEN = (H + 2) * PADW          # 6724
BN_COUNT = float(B * NPIX)       # 102400


def _rsqrt(nc, pool, a, p, cols, tag):
    """rstd = 1/sqrt(a) on a [p, cols] fp32 SBUF AP via bit trick + 2 Newton
    iterations (all DVE; ACT Sqrt would force an act-table swap vs Silu)."""
    sh = pool.tile([p, cols], I32, tag=f"{tag}sh")
    nc.vector.tensor_scalar(out=sh, in0=a.bitcast(I32), scalar1=1, scalar2=None,
                            op0=ALU.logical_shift_right)
    yn = pool.tile([p, cols], I32, tag=f"{tag}yn")
    nc.vector.tensor_scalar(out=yn, in0=sh, scalar1=-1, scalar2=None,
                            op0=ALU.bitwise_xor)
    yi = pool.tile([p, cols], I32, tag=f"{tag}yi")
    nc.vector.tensor_scalar(out=yi, in0=yn, scalar1=0x5F3759DF + 1, scalar2=None,
                            op0=ALU.add)
    y = yi.bitcast(F32)
    for it in range(2):
        u = pool.tile([p, cols], F32, tag=f"{tag}u{it}")
        nc.vector.tensor_tensor(out=u, in0=y, in1=y, op=ALU.mult)
        w = pool.tile([p, cols], F32, tag=f"{tag}w{it}")
        nc.vector.tensor_tensor(out=w, in0=u, in1=a, op=ALU.mult)
        z = pool.tile([p, cols], F32, tag=f"{tag}z{it}")
        nc.vector.tensor_scalar(out=z, in0=w, scalar1=-0.5, scalar2=1.5,
                                op0=ALU.mult, op1=ALU.add)
        y2 = pool.tile([p, cols], F32, tag=f"{tag}y{it}")
        nc.vector.tensor_tensor(out=y2, in0=y, in1=z, op=ALU.mult)
        y = y2
    return y


def build_module(h_bf16=True, sim_silu=False):
    nc = bacc.Bacc("TRN2", target_bir_lowering=False, debug=False,
                   num_devices=NCORES)
    HDT = BF16 if h_bf16 else F32
    WBDT = BF16 if h_bf16 else F32R

    x_d = nc.dram_tensor("x", [SPC, C, NPIX], F32R, kind="ExternalInput").ap()
    swT_d = nc.dram_tensor("swT", [C, 9, C], F32R, kind="ExternalInput").ap()
    waT_d = nc.dram_tensor("waT", [SPC, C, HID], F32R, kind="ExternalInput").ap()
    wbT_d = nc.dram_tensor("wbT", [SPC, C, 2, C], WBDT, kind="ExternalInput").ap()
    p1_d = nc.dram_tensor("p1", [SPC, C, 4], F32, kind="ExternalInput").ap()
    p2_d = nc.dram_tensor("p2", [SPC, C, 4], F32, kind="ExternalInput").ap()
    bnp_d = nc.dram_tensor("bnp", [C, 2], F32, kind="ExternalInput").ap()
    gm1_d = nc.dram_tensor("gm1", [C, 4], F32, kind="ExternalInput").ap()
    gb1_d = nc.dram_tensor("gb1", [4, C], F32, kind="ExternalInput").ap()
    gm2_d = nc.dram_tensor("gm2", [C, 8], F32, kind="ExternalInput").ap()
    gb2_d = nc.dram_tensor("gb2", [8, C], F32, kind="ExternalInput").ap()
    zpad_d = nc.dram_tensor("zpad", [C, PADW], F32R, kind="ExternalInput").ap()
    out_d = nc.dram_tensor("out", [SPC, C, NPIX], F32, kind="ExternalOutput").ap()

    with tile.TileContext(nc) as tc:
        _body(nc, tc, locals(), h_bf16=h_bf16, sim_silu=sim_silu, HDT=HDT)
    nc.compile()
    return nc


def _silu_apply(nc, pool, out_ap, in_ap, scale, bias, sim_silu, tag):
    """out = silu(in*scale + bias); CoreSim lacks Silu so emulate there."""
    if not sim_silu:
        nc.scalar.activation(out=out_ap, in_=in_ap, func=AF.Silu,
                             bias=bias, scale=scale)
        return
    p, n = in_ap.shape[0], in_ap.free_size()
    sg = pool.tile([p, n], F32, tag="simsg", name=f"{tag}sg")
    nc.scalar.activation(out=sg, in_=in_ap, func=AF.Sigmoid,
                         bias=bias, scale=scale)
    zz = pool.tile([p, n], F32, tag="simzz", name=f"{tag}zz")
    nc.vector.tensor_scalar(out=zz, in0=in_ap, scalar1=scale, scalar2=bias,
                            op0=ALU.mult, op1=ALU.add)
    nc.vector.tensor_tensor(out=out_ap, in0=sg, in1=zz, op=ALU.mult)


def _body(nc, tc, d, h_bf16, sim_silu, HDT):
    x_d, swT_d, waT_d, wbT_d = d["x_d"], d["swT_d"], d["waT_d"], d["wbT_d"]
    zpad_d = d["zpad_d"]
    p1_d, p2_d, bnp_d = d["p1_d"], d["p2_d"], d["bnp_d"]
    gm1_d, gb1_d, gm2_d, gb2_d = d["gm1_d"], d["gb1_d"], d["gm2_d"], d["gb2_d"]
    out_d = d["out_d"]

    import contextlib
    ctx = contextlib.ExitStack()
    with ctx:
        wp = ctx.enter_context(tc.tile_pool(name="wp", bufs=1))
        wp2 = ctx.enter_context(tc.tile_pool(name="wp2", bufs=SPC))
        sp = ctx.enter_context(tc.tile_pool(name="sp", bufs=SPC))
        h1p = ctx.enter_context(tc.tile_pool(name="h1p", bufs=SPC))
        h2p = ctx.enter_context(tc.tile_pool(name="h2p", bufs=SPC))
        st = ctx.enter_context(tc.tile_pool(name="st", bufs=SPC))
        st1 = ctx.enter_context(tc.tile_pool(name="st1", bufs=1))
        cps = ctx.enter_context(tc.tile_pool(name="cps", bufs=6, space="PSUM"))
        mps = ctx.enter_context(tc.tile_pool(name="mps", bufs=2, space="PSUM"))
        dram = ctx.enter_context(tc.tile_pool(name="dram", bufs=1, space="DRAM"))
        simp = (ctx.enter_context(tc.tile_pool(name="simp", bufs=1))
                if sim_silu else None)

        # ---- persistent weights / params ----
        swT = wp.tile([C, 9, C], F32R, tag="swT")
        nc.sync.dma_start(out=swT, in_=swT_d)
        gm1 = wp.tile([C, 4], F32, tag="gm1")
        nc.sync.dma_start(out=gm1, in_=gm1_d)
        gb1 = wp.tile([4, C], F32, tag="gb1")
        nc.sync.dma_start(out=gb1, in_=gb1_d)
        gm2 = wp.tile([C, 8], F32, tag="gm2")
        nc.sync.dma_start(out=gm2, in_=gm2_d)
        gb2 = wp.tile([8, C], F32, tag="gb2")
        nc.sync.dma_start(out=gb2, in_=gb2_d)
        bnp = wp.tile([C, 2], F32, tag="bnp")
        nc.sync.dma_start(out=bnp, in_=bnp_d)

        waT, wbT, p1, p2 = [], [], [], []
        for s in range(SPC):
            waT.append(wp2.tile([C, HID], F32R, tag="waT", name=f"waT{s}"))
            nc.sync.dma_start(out=waT[s], in_=waT_d[s])
            wbT.append(wp2.tile([C, 2, C], wbT_d.dtype, tag="wbT", name=f"wbT{s}"))
            nc.sync.dma_start(out=wbT[s], in_=wbT_d[s])
            p1.append(wp2.tile([C, 4], F32, tag="p1", name=f"p1_{s}"))
            nc.sync.dma_start(out=p1[s], in_=p1_d[s])
            p2.append(wp2.tile([C, 4], F32, tag="p2", name=f"p2_{s}"))
            nc.sync.dma_start(out=p2[s], in_=p2_d[s])

        # ---- sample-persistent big tiles ----
        s_sb = [sp.tile([C, NPIX], F32, tag="s", name=f"s{i}") for i in range(SPC)]
        ssum = st1.tile([C, SPC * NTILES], F32, tag="ssum")
        ssq = st1.tile([C, SPC * NTILES], F32, tag="ssq")

        # =========== phase 1: conv3x3 + conv1 (uses padded x) ===========
        xpp = ctx.enter_context(tc.tile_pool(name="xpadp", bufs=SPC, side="right"))
        if True:
            xpad = []
            for s in range(SPC):
                xp = xpp.tile([C, PADLEN], F32R, tag="xpad", name=f"xpad{s}")
                xpad.append(xp)
                xr = xp.rearrange("p (r c) -> p r c", c=PADW)
                nc.sync.dma_start(out=xr[:, 0:1, :], in_=zpad_d)
                nc.sync.dma_start(out=xr[:, H + 1:H + 2, :], in_=zpad_d)
                nc.sync.dma_start(out=xr[:, :, 0:1], in_=zpad_d)
                nc.sync.dma_start(out=xr[:, :, PADW - 1:PADW], in_=zpad_d)
                nc.sync.dma_start(
                    out=xr[:, 1:H + 1, 1:W + 1],
                    in_=x_d[s].rearrange("p (h w) -> p h w", w=W),
                )

            # ---- conv3x3: s = shared conv, BN partial sums via ACT accum ----
            for s in range(SPC):
                xr = xpad[s].rearrange("p (r c) -> p r c", c=PADW)
                for t in range(NTILES):
                    r0 = t * ROWS_PER_TILE
                    ps = cps.tile([C, TILE_N], F32, tag="cps")
                    for tap in range(9):
                        dy, dx = divmod(tap, 3)
                        rhs = xr[:, r0 + dy:r0 + dy + ROWS_PER_TILE, dx:dx + W]
                        nc.tensor.matmul(out=ps, lhsT=swT[:, tap, :], rhs=rhs,
                                         start=(tap == 0), stop=(tap == 8))
                    col = s * NTILES + t
                    nc.scalar.activation(
                        out=s_sb[s][:, t * TILE_N:(t + 1) * TILE_N], in_=ps,
                        func=AF.Copy, accum_out=ssum[:, col:col + 1])
                    nc.scalar.activation(out=ps, in_=ps, func=AF.Square,
                                         accum_out=ssq[:, col:col + 1])

            # ---- BN partial -> DRAM bounce -> AllReduce ----
            bnpart = st1.tile([C, 2], F32, tag="bnpart")
            nc.vector.reduce_sum(out=bnpart[:, 0:1], in_=ssum, axis=mybir.AxisListType.X)
            nc.vector.reduce_sum(out=bnpart[:, 1:2], in_=ssq, axis=mybir.AxisListType.X)
            bn_in = dram.tile([C, 2], F32, tag="bn_in")
            bn_out = dram.tile([C, 2], F32, tag="bn_out", addr_space="Shared")
            nc.sync.dma_start(out=bn_in, in_=bnpart)
            nc.gpsimd.collective_compute(
                "AllReduce", ALU.add,
                replica_groups=[list(range(NCORES))],
                ins=[bn_in.opt()], outs=[bn_out.opt()],
            )
            bnagg = st1.tile([C, 2], F32, tag="bnagg")
            nc.sync.dma_start(out=bnagg, in_=bn_out)

            # BN finalize: k = gamma*rsqrt(var+eps), c = beta - mean*k
            bnm = st1.tile([C, 2], F32, tag="bnm")  # (mean, E2)
            nc.vector.tensor_scalar(out=bnm, in0=bnagg, scalar1=1.0 / BN_COUNT,
                                    scalar2=None, op0=ALU.mult)
            bnv = st1.tile([C, 1], F32, tag="bnv")
            nc.vector.tensor_tensor(out=bnv, in0=bnm[:, 0:1], in1=bnm[:, 0:1],
                                    op=ALU.mult)
            nc.vector.tensor_tensor(out=bnv, in0=bnm[:, 1:2], in1=bnv,
                                    op=ALU.subtract)
            nc.vector.tensor_scalar(out=bnv, in0=bnv, scalar1=EPS, scalar2=None,
                                    op0=ALU.add)
            bnr = _rsqrt(nc, st1, bnv, C, 1, "bnr")
            bn_k = st1.tile([C, 1], F32, tag="bn_k")
            nc.vector.tensor_tensor(out=bn_k, in0=bnr, in1=bnp[:, 0:1], op=ALU.mult)
            bn_c = st1.tile([C, 1], F32, tag="bn_c")
            nc.vector.tensor_tensor(out=bn_c, in0=bnm[:, 0:1], in1=bn_k, op=ALU.mult)
            nc.vector.tensor_tensor(out=bn_c, in0=bnp[:, 1:2], in1=bn_c,
                                    op=ALU.subtract)

            # ---- conv1: h1 = x @ waT (two 128-wide halves), GN1, SiLU ----
            h1_sb, k1s, c1s = [], [], []
            for s in range(SPC):
                xr = xpad[s].rearrange("p (r c) -> p r c", c=PADW)
                h1 = h1p.tile([C, 2, NPIX], HDT, tag="h1", name=f"h1_{s}")
                h1_sb.append(h1)
                h1st = st.tile([C, 2, NTILES, 6], F32, tag="h1st")
                for t in range(NTILES):
                    r0 = t * ROWS_PER_TILE
                    rhs = xr[:, r0 + 1:r0 + 1 + ROWS_PER_TILE, 1:1 + W]
                    for half in range(2):
                        ps = cps.tile([C, TILE_N], F32, tag="cps")
                        nc.tensor.matmul(
                            out=ps, lhsT=waT[s][:, half * C:(half + 1) * C],
                            rhs=rhs, start=True, stop=True)
                        nc.vector.bn_stats(out=h1st[:, half, t, :], in_=ps)
                        dst = h1[:, half, t * TILE_N:(t + 1) * TILE_N]
                        if half == 0:
                            nc.scalar.activation(out=dst, in_=ps, func=AF.Copy)
                        else:
                            nc.vector.tensor_copy(out=dst, in_=ps)

                # GN1 chain: per-half groups of 32 channels
                mv = st.tile([C, 2, 2], F32, tag="gn1mv")
                for half in range(2):
                    nc.vector.bn_aggr(out=mv[:, half, :], in_=h1st[:, half])
                # rhs cols: (m0, q0, m1, q1), q = m*m + var
                rhs_t = st.tile([C, 4], F32, tag="gn1rhs")
                mvv = mv.rearrange("p a b -> p (a b)")  # (m0, v0, m1, v1)
                nc.vector.tensor_copy(out=rhs_t[:, 0:4:2], in_=mvv[:, 0:4:2])
                nc.vector.tensor_tensor(out=rhs_t[:, 1:4:2], in0=mvv[:, 0:4:2],
                                        in1=mvv[:, 0:4:2], op=ALU.mult)
                nc.vector.tensor_tensor(out=rhs_t[:, 1:4:2], in0=rhs_t[:, 1:4:2],
                                        in1=mvv[:, 1:4:2], op=ALU.add)
                psg = mps.tile([4, 4], F32, tag="mps")
                nc.tensor.matmul(out=psg, lhsT=gm1, rhs=rhs_t, start=True, stop=True)
                gsb = st.tile([4, 4], F32, tag="gn1gsb")
                nc.vector.tensor_copy(out=gsb, in_=psg)
                gv = st.tile([4, 2], F32, tag="gn1gv")
                nc.vector.tensor_tensor(out=gv, in0=gsb[:, 0:4:2], in1=gsb[:, 0:4:2],
                                        op=ALU.mult)
                nc.vector.tensor_tensor(out=gv, in0=gsb[:, 1:4:2], in1=gv,
                                        op=ALU.subtract)
                nc.vector.tensor_scalar(out=gv, in0=gv, scalar1=EPS, scalar2=None,
                                        op0=ALU.add)
                gr = _rsqrt(nc, st, gv, 4, 2, "gn1r")
                rhs2 = st.tile([4, 4], F32, tag="gn1rhs2")
                nc.vector.tensor_copy(out=rhs2[:, 0:4:2], in_=gsb[:, 0:4:2])
                nc.vector.tensor_copy(out=rhs2[:, 1:4:2], in_=gr)
                psb = mps.tile([C, 4], F32, tag="mps")
                nc.tensor.matmul(out=psb, lhsT=gb1, rhs=rhs2, start=True, stop=True)
                mrc = st.tile([C, 4], F32, tag="gn1mrc")  # (m_c0, r_c0, m_c1, r_c1)
                nc.vector.tensor_copy(out=mrc, in_=psb)
                k1 = st.tile([C, 2], F32, tag="gn1k")
                nc.vector.tensor_tensor(out=k1, in0=mrc[:, 1:4:2], in1=p1[s][:, 0:2],
                                        op=ALU.mult)
                c1 = st.tile([C, 2], F32, tag="gn1c")
                nc.vector.tensor_tensor(out=c1, in0=mrc[:, 0:4:2], in1=k1, op=ALU.mult)
                nc.vector.tensor_tensor(out=c1, in0=p1[s][:, 2:4], in1=c1,
                                        op=ALU.subtract)
                k1s.append(k1)
                c1s.append(c1)

                # a1 = silu(h1*k + c) in place, per half, chunked
                for half in range(2):
                    for ch in range(NCHUNKS):
                        ap = h1[:, half, ch * CHUNK:(ch + 1) * CHUNK]
                        _silu_apply(nc, simp, ap, ap, k1[:, half:half + 1],
                                    c1[:, half:half + 1], sim_silu, "gn1a")

        # =========== phase 2: conv2, GN2, BN apply, combine ===========
        if True:
            late = xpp
            h2_sb, css, m2ps = [], [], []
            for s in range(SPC):
                h2 = h2p.tile([C, NPIX], HDT, tag="h2", name=f"h2_{s}")
                h2_sb.append(h2)
                h2st = st.tile([C, NTILES, 6], F32, tag="h2st")
                for t in range(NTILES):
                    ps = cps.tile([C, TILE_N], F32, tag="cps")
                    for half in range(2):
                        nc.tensor.matmul(
                            out=ps, lhsT=wbT[s][:, half, :],
                            rhs=h1_sb[s][:, half, t * TILE_N:(t + 1) * TILE_N],
                            start=(half == 0), stop=(half == 1))
                    nc.vector.bn_stats(out=h2st[:, t, :], in_=ps)
                    dst = h2[:, t * TILE_N:(t + 1) * TILE_N]
                    if s == 0:
                        nc.scalar.activation(out=dst, in_=ps, func=AF.Copy)
                    else:
                        nc.vector.tensor_copy(out=dst, in_=ps)

                # GN2 chain: 8 groups of 16 channels
                mv2 = st.tile([C, 2], F32, tag="gn2mv")
                nc.vector.bn_aggr(out=mv2, in_=h2st)
                rhs_t = st.tile([C, 2], F32, tag="gn2rhs")
                nc.vector.tensor_copy(out=rhs_t[:, 0:1], in_=mv2[:, 0:1])
                nc.vector.tensor_tensor(out=rhs_t[:, 1:2], in0=mv2[:, 0:1],
                                        in1=mv2[:, 0:1], op=ALU.mult)
                nc.vector.tensor_tensor(out=rhs_t[:, 1:2], in0=rhs_t[:, 1:2],
                                        in1=mv2[:, 1:2], op=ALU.add)
                psg = mps.tile([8, 2], F32, tag="mps")
                nc.tensor.matmul(out=psg, lhsT=gm2, rhs=rhs_t, start=True, stop=True)
                gsb = st.tile([8, 2], F32, tag="gn2gsb")
                nc.vector.tensor_copy(out=gsb, in_=psg)
                gv = st.tile([8, 1], F32, tag="gn2gv")
                nc.vector.tensor_tensor(out=gv, in0=gsb[:, 0:1], in1=gsb[:, 0:1],
                                        op=ALU.mult)
                nc.vector.tensor_tensor(out=gv, in0=gsb[:, 1:2], in1=gv,
                                        op=ALU.subtract)
                nc.vector.tensor_scalar(out=gv, in0=gv, scalar1=EPS, scalar2=None,
                                        op0=ALU.add)
                gr = _rsqrt(nc, st, gv, 8, 1, "gn2r")
                rhs2 = st.tile([8, 2], F32, tag="gn2rhs2")
                nc.vector.tensor_copy(out=rhs2[:, 0:1], in_=gsb[:, 0:1])
                nc.vector.tensor_copy(out=rhs2[:, 1:2], in_=gr)
                psb = mps.tile([C, 2], F32, tag="mps")
                nc.tensor.matmul(out=psb, lhsT=gb2, rhs=rhs2, start=True, stop=True)
                mrc = st.tile([C, 2], F32, tag="gn2mrc")  # (m_c, r_c)
                nc.vector.tensor_copy(out=mrc, in_=psb)
                # cs = r*g2*coef ; m2' = m2 - b2c/cs
                cs = st.tile([C, 1], F32, tag="gn2cs")
                nc.vector.tensor_tensor(out=cs, in0=mrc[:, 1:2], in1=p2[s][:, 0:1],
                                        op=ALU.mult)
                nc.vector.tensor_tensor(out=cs, in0=cs, in1=p2[s][:, 2:3],
                                        op=ALU.mult)
                rcs = st.tile([C, 1], F32, tag="gn2rcs")
                nc.vector.reciprocal(out=rcs, in_=cs)
                m2p = st.tile([C, 1], F32, tag="gn2m2p")
                nc.vector.tensor_tensor(out=m2p, in0=p2[s][:, 1:2], in1=rcs,
                                        op=ALU.mult)
                nc.vector.tensor_tensor(out=m2p, in0=mrc[:, 0:1], in1=m2p,
                                        op=ALU.subtract)
                css.append(cs)
                m2ps.append(m2p)

            # ---- BN apply + combine, chunked ----
            for s in range(SPC):
                for ch in range(NCHUNKS):
                    ap = s_sb[s][:, ch * CHUNK:(ch + 1) * CHUNK]
                    _silu_apply(nc, simp, ap, ap, bn_k, bn_c, sim_silu, "bna")
                rpc = CHUNK // W  # rows per chunk
                xr2 = xpad[s].rearrange("p (r c) -> p r c", c=PADW)
                for ch in range(NCHUNKS):
                    sl = slice(ch * CHUNK, (ch + 1) * CHUNK)
                    xint = xr2[:, 1 + ch * rpc:1 + (ch + 1) * rpc, 1:1 + W]
                    outc = late.tile([C, CHUNK], F32, tag="outc")
                    # O3: out = x*pass + silu(bn(s))     (DVE)
                    nc.vector.scalar_tensor_tensor(
                        out=outc, in0=xint.bitcast(F32), scalar=p2[s][:, 3:4],
                        in1=s_sb[s][:, sl], op0=ALU.mult, op1=ALU.add)
                    # O4: h2n = (h2 - m2')*cs             (DVE, bf16 2x)
                    h2n = late.tile([C, CHUNK], HDT, tag="h2n")
                    nc.vector.tensor_scalar(
                        out=h2n, in0=h2_sb[s][:, sl], scalar1=m2ps[s],
                        scalar2=css[s], op0=ALU.subtract, op1=ALU.mult)
                    # O5: out += h2n                      (Pool)
                    nc.gpsimd.tensor_tensor(out=outc, in0=outc, in1=h2n,
                                            op=ALU.add)
                    nc.sync.dma_start(out=out_d[s][:, sl], in_=outc)


# ---------------- host side ----------------

_module_cache = {}


def _get_module(h_bf16=True, sim_silu=False):
    key = (h_bf16, sim_silu)
    if key not in _module_cache:
        _module_cache[key] = build_module(h_bf16=h_bf16, sim_silu=sim_silu)
    return _module_cache[key]


def make_in_maps(x, weights, indices, shared_w, bn_gamma, bn_beta,
                 w1, g1, b1, w2, g2, b2, h_bf16=True):
    """Shard + route on host: returns per-core input dicts."""
    import ml_dtypes
    wb_np = ml_dtypes.bfloat16 if h_bf16 else np.float32

    x = np.asarray(x, np.float32)
    weights = np.asarray(weights, np.float32)
    indices = np.asarray(indices).astype(np.int64)
    shared_w = np.asarray(shared_w, np.float32)
    w1 = np.asarray(w1, np.float32)
    w2 = np.asarray(w2, np.float32)
    g1 = np.asarray(g1, np.float32)
    b1 = np.asarray(b1, np.float32)
    g2 = np.asarray(g2, np.float32)
    b2 = np.asarray(b2, np.float32)

    # shared conv weights -> [C_in, 9, C_out]
    swT = np.ascontiguousarray(shared_w.transpose(1, 2, 3, 0).reshape(C, 9, C))
    bnp = np.stack([np.asarray(bn_gamma, np.float32),
                    np.asarray(bn_beta, np.float32)], axis=1)

    gm1 = np.zeros((C, 4), np.float32)
    for g in range(4):
        gm1[g * 32:(g + 1) * 32, g] = 1.0 / 32.0
    gb1 = np.zeros((4, C), np.float32)
    for g in range(4):
        gb1[g, g * 32:(g + 1) * 32] = 1.0
    gm2 = np.zeros((C, 8), np.float32)
    for g in range(8):
        gm2[g * 16:(g + 1) * 16, g] = 1.0 / 16.0
    gb2 = np.zeros((8, C), np.float32)
    for g in range(8):
        gb2[g, g * 16:(g + 1) * 16] = 1.0

    in_maps = []
    for core in range(NCORES):
        sl = slice(core * SPC, (core + 1) * SPC)
        xs = np.ascontiguousarray(x[sl].reshape(SPC, C, NPIX))
        waT = np.zeros((SPC, C, HID), np.float32)
        wbT = np.zeros((SPC, C, 2, C), np.float32)
        p1 = np.zeros((SPC, C, 4), np.float32)
        p2 = np.zeros((SPC, C, 4), np.float32)
        for s in range(SPC):
            b_idx = core * SPC + s
            idx = int(indices[b_idx])
            e = max(idx - 1, 0)
            coef = float(weights[b_idx]) if idx > 0 else 0.0
            pass_c = float(weights[b_idx]) if idx == 0 else 0.0
            waT[s] = w1[e].T                       # [C, HID]
            w2T = w2[e].T                          # [HID, C]
            wbT[s, :, 0, :] = w2T[:C]
            wbT[s, :, 1, :] = w2T[C:]
            p1[s, :, 0] = g1[e][:C]
            p1[s, :, 1] = g1[e][C:]
            p1[s, :, 2] = b1[e][:C]
            p1[s, :, 3] = b1[e][C:]
            p2[s, :, 0] = g2[e]
            p2[s, :, 1] = b2[e] * coef
            p2[s, :, 2] = max(coef, 1e-30)
            p2[s, :, 3] = pass_c
        in_maps.append(dict(
            x=xs, swT=swT, waT=waT, wbT=wbT.astype(wb_np), p1=p1, p2=p2,
            bnp=bnp, gm1=gm1, gb1=gb1, gm2=gm2, gb2=gb2,
            zpad=np.zeros((C, PADW), np.float32),
        ))
    return in_maps


def kernel(**inputs) -> np.ndarray:
    from concourse import bass_utils

    h_bf16 = os.environ.get("MOE_H_BF16", "1") == "1"
    nc = _get_module(h_bf16=h_bf16, sim_silu=False)
    in_maps = make_in_maps(h_bf16=h_bf16, **inputs)
    res = bass_utils.run_bass_kernel_spmd(
        nc, in_maps, core_ids=list(range(NCORES)),
        trace=os.environ.get("MOE_TRACE", "0") == "1",
    )
    if res.exec_time_ns is not None:
        print(f"HW exec time: {res.exec_time_ns} ns")
    out = np.concatenate([r["out"] for r in res.results], axis=0)
    return out.reshape(B, C, H, W).astype(np.float32)


# revision 14
# speedup vs baseline: 1.4679x; 1.4679x over previous
"""Trainium2 Bass kernel for nn_DecoupledMoEContainer_52596169507001.

Strategy: data-parallel over batch, 2 samples per core across 8 NeuronCores.
Host-side routing: per-sample expert weights are gathered by `indices` so the
device program is uniform (the NEFF does not depend on input values).

Per sample on device:
  shared: s = conv3x3(x)            -> 9 accumulated fp32r matmuls per row-tile
          BN (batch stats)          -> per-channel sum/sumsq, cross-core
                                       AllReduce of [128,2], then SiLU apply
  expert: h1 = conv1x1 (C->HID)     -> 2 fp32r matmuls per row-tile
          GN(8)+SiLU                -> bn_stats + tiny mask-matmuls + rsqrt
          h2 = conv1x1 (HID->C)     -> 2 accumulated matmuls per row-tile
          GN(8)                     -> folded into final combine
  out = silu(bn(s)) + pass*x + (h2 - m2')*cs      (routing coefs from host)
"""

import os

import numpy as np

import concourse.bacc as bacc
import concourse.bass as bass
import concourse.mybir as mybir
import concourse.tile as tile

F32 = mybir.dt.float32
F32R = mybir.dt.float32r
BF16 = mybir.dt.bfloat16
I32 = mybir.dt.int32
AF = mybir.ActivationFunctionType
ALU = mybir.AluOpType

# Problem shapes (hardcoded per spec)
B, C, HID, H, W = 16, 128, 256, 80, 80
NPIX = H * W                     # 6400
NCORES = 8
SPC = B // NCORES                # samples per core = 2
EPS = 1e-5

ROWS_PER_TILE = 5
NTILES = H // ROWS_PER_TILE      # 16
TILE_N = ROWS_PER_TILE * W       # 400
CHUNK = 800
NCHUNKS = NPIX // CHUNK          # 8
PADW = W + 2                     # 82
PADLEN = (H + 2) * PADW          # 6724
BN_COUNT = float(B * NPIX)       # 102400


def _rsqrt(nc, pool, a, p, cols, tag):
    """rstd = 1/sqrt(a) on a [p, cols] fp32 SBUF AP via bit trick + 2 Newton
    iterations (all DVE; ACT Sqrt would force an act-table swap vs Silu)."""
    sh = pool.tile([p, cols], I32, tag=f"{tag}sh")
    nc.vector.tensor_scalar(out=sh, in0=a.bitcast(I32), scalar1=1, scalar2=None,
                            op0=ALU.logical_shift_right)
    yn = pool.tile([p, cols], I32, tag=f"{tag}yn")
    nc.vector.tensor_scalar(out=yn, in0=sh, scalar1=-1, scalar2=None,
                            op0=ALU.bitwise_xor)
    yi = pool.tile([p, cols], I32, tag=f"{tag}yi")
    nc.vector.tensor_scalar(out=yi, in0=yn, scalar1=0x5F3759DF + 1, scalar2=None,
                            op0=ALU.add)
    y = yi.bitcast(F32)
    for it in range(2):
        u = pool.tile([p, cols], F32, tag=f"{tag}u{it}")
        nc.vector.tensor_tensor(out=u, in0=y, in1=y, op=ALU.mult)
        w = pool.tile([p, cols], F32, tag=f"{tag}w{it}")
        nc.vector.tensor_tensor(out=w, in0=u, in1=a, op=ALU.mult)
        z = pool.tile([p, cols], F32, tag=f"{tag}z{it}")
        nc.vector.tensor_scalar(out=z, in0=w, scalar1=-0.5, scalar2=1.5,
                                op0=ALU.mult, op1=ALU.add)
        y2 = pool.tile([p, cols], F32, tag=f"{tag}y{it}")
        nc.vector.tensor_tensor(out=y2, in0=y, in1=z, op=ALU.mult)
        y = y2
    return y


def build_module(h_bf16=True, sim_silu=False):
    nc = bacc.Bacc("TRN2", target_bir_lowering=False, debug=False,
                   num_devices=NCORES)
    HDT = BF16 if h_bf16 else F32
    WBDT = BF16 if h_bf16 else F32R

    x_d = nc.dram_tensor("x", [SPC, C, NPIX], F32R, kind="ExternalInput").ap()
    swT_d = nc.dram_tensor("swT", [C, 9, C], F32R, kind="ExternalInput").ap()
    waT_d = nc.dram_tensor("waT", [SPC, C, HID], F32R, kind="ExternalInput").ap()
    wbT_d = nc.dram_tensor("wbT", [SPC, C, 2, C], WBDT, kind="ExternalInput").ap()
    p1_d = nc.dram_tensor("p1", [SPC, C, 4], F32, kind="ExternalInput").ap()
    p2_d = nc.dram_tensor("p2", [SPC, C, 4], F32, kind="ExternalInput").ap()
    bnp_d = nc.dram_tensor("bnp", [C, 2], F32, kind="ExternalInput").ap()
    gm1_d = nc.dram_tensor("gm1", [C, 4], F32, kind="ExternalInput").ap()
    gb1_d = nc.dram_tensor("gb1", [4, C], F32, kind="ExternalInput").ap()
    gm2_d = nc.dram_tensor("gm2", [C, 8], F32, kind="ExternalInput").ap()
    gb2_d = nc.dram_tensor("gb2", [8, C], F32, kind="ExternalInput").ap()
    zpad_d = nc.dram_tensor("zpad", [C, PADW], F32R, kind="ExternalInput").ap()
    out_d = nc.dram_tensor("out", [SPC, C, NPIX], F32, kind="ExternalOutput").ap()

    with tile.TileContext(nc) as tc:
        _body(nc, tc, locals(), h_bf16=h_bf16, sim_silu=sim_silu, HDT=HDT)
    nc.compile()
    return nc


def _silu_apply(nc, pool, out_ap, in_ap, scale, bias, sim_silu, tag):
    """out = silu(in*scale + bias); CoreSim lacks Silu so emulate there."""
    if not sim_silu:
        nc.scalar.activation(out=out_ap, in_=in_ap, func=AF.Silu,
                             bias=bias, scale=scale)
        return
    p, n = in_ap.shape[0], in_ap.free_size()
    for q in range(2):
        i_ap = in_ap[:, q * (n // 2):(q + 1) * (n // 2)]
        o_ap = out_ap[:, q * (n // 2):(q + 1) * (n // 2)]
        sg = pool.tile([p, n // 2], F32, tag="simsg", name=f"{tag}sg{q}")
        nc.scalar.activation(out=sg, in_=i_ap, func=AF.Sigmoid,
                             bias=bias, scale=scale)
        zz = pool.tile([p, n // 2], F32, tag="simzz", name=f"{tag}zz{q}")
        nc.vector.tensor_scalar(out=zz, in0=i_ap, scalar1=scale, scalar2=bias,
                                op0=ALU.mult, op1=ALU.add)
        nc.vector.tensor_tensor(out=o_ap, in0=sg, in1=zz, op=ALU.mult)



def _conv1(nc, s, xr, cps, st, mps, h1p, waT, p1, gm1, gb1,
           h1_sb, k1s, c1s, HDT, sim_silu, simp, st1):
    """conv1 (C->HID) + GN1 stats/chain + SiLU apply for sample s."""
    h1 = h1p.tile([C, 2, NPIX], HDT, tag="h1", name=f"h1_{s}")
    h1_sb.append(h1)
    h1st = st.tile([C, 2, NTILES, 6], F32, tag="h1st", name=f"h1st{s}")
    for t in range(NTILES):
        r0 = t * ROWS_PER_TILE
        rhs = xr[:, r0 + 1:r0 + 1 + ROWS_PER_TILE, 1:1 + W]
        for half in range(2):
            ps = cps.tile([C, TILE_N], F32, tag="cps", name=f"c1ps{s}_{t}_{half}")
            nc.tensor.matmul(
                out=ps, lhsT=waT[s][:, half * C:(half + 1) * C],
                rhs=rhs, start=True, stop=True)
            nc.vector.bn_stats(out=h1st[:, half, t, :], in_=ps)
            dst = h1[:, half, t * TILE_N:(t + 1) * TILE_N]
            if half == 0:
                nc.scalar.activation(out=dst, in_=ps, func=AF.Copy)
            else:
                nc.vector.tensor_copy(out=dst, in_=ps)

    # GN1 chain: per-half groups of 32 channels
    mv = st.tile([C, 2, 2], F32, tag="gn1mv", name=f"gn1mv{s}")
    for half in range(2):
        nc.vector.bn_aggr(out=mv[:, half, :], in_=h1st[:, half])
    # rhs cols: (m0, q0, m1, q1), q = m*m + var
    rhs_t = st.tile([C, 4], F32, tag="gn1rhs", name=f"gn1rhs{s}")
    mvv = mv.rearrange("p a b -> p (a b)")  # (m0, v0, m1, v1)
    nc.vector.tensor_copy(out=rhs_t[:, 0:4:2], in_=mvv[:, 0:4:2])
    nc.vector.tensor_tensor(out=rhs_t[:, 1:4:2], in0=mvv[:, 0:4:2],
                            in1=mvv[:, 0:4:2], op=ALU.mult)
    nc.vector.tensor_tensor(out=rhs_t[:, 1:4:2], in0=rhs_t[:, 1:4:2],
                            in1=mvv[:, 1:4:2], op=ALU.add)
    psg = mps.tile([4, 4], F32, tag="mps", name=f"gn1psg{s}")
    nc.tensor.matmul(out=psg, lhsT=gm1, rhs=rhs_t, start=True, stop=True)
    gsb = st.tile([4, 4], F32, tag="gn1gsb", name=f"gn1gsb{s}")
    nc.vector.tensor_copy(out=gsb, in_=psg)
    gv = st.tile([4, 2], F32, tag="gn1gv", name=f"gn1gv{s}")
    nc.vector.tensor_tensor(out=gv, in0=gsb[:, 0:4:2], in1=gsb[:, 0:4:2],
                            op=ALU.mult)
    nc.vector.tensor_tensor(out=gv, in0=gsb[:, 1:4:2], in1=gv,
                            op=ALU.subtract)
    nc.vector.tensor_scalar(out=gv, in0=gv, scalar1=EPS, scalar2=None,
                            op0=ALU.add)
    gr = _rsqrt(nc, st, gv, 4, 2, "gn1r")
    rhs2 = st.tile([4, 4], F32, tag="gn1rhs2", name=f"gn1rhs2{s}")
    nc.vector.tensor_copy(out=rhs2[:, 0:4:2], in_=gsb[:, 0:4:2])
    nc.vector.tensor_copy(out=rhs2[:, 1:4:2], in_=gr)
    psb = mps.tile([C, 4], F32, tag="mps", name=f"gn1psb{s}")
    nc.tensor.matmul(out=psb, lhsT=gb1, rhs=rhs2, start=True, stop=True)
    mrc = st.tile([C, 4], F32, tag="gn1mrc", name=f"gn1mrc{s}")
    nc.vector.tensor_copy(out=mrc, in_=psb)
    k1 = st.tile([C, 2], F32, tag="gn1k", name=f"gn1k{s}")
    nc.vector.tensor_tensor(out=k1, in0=mrc[:, 1:4:2], in1=p1[s][:, 0:2],
                            op=ALU.mult)
    c1 = st.tile([C, 2], F32, tag="gn1c", name=f"gn1c{s}")
    nc.vector.tensor_tensor(out=c1, in0=mrc[:, 0:4:2], in1=k1, op=ALU.mult)
    nc.vector.tensor_tensor(out=c1, in0=p1[s][:, 2:4], in1=c1,
                            op=ALU.subtract)
    k1s.append(k1)
    c1s.append(c1)

    # a1 = silu(h1*k + c) in place, per half, chunked
    for half in range(2):
        for ch in range(NCHUNKS):
            ap = h1[:, half, ch * CHUNK:(ch + 1) * CHUNK]
            _silu_apply(nc, simp, ap, ap, k1[:, half:half + 1],
                        c1[:, half:half + 1], sim_silu, "gn1a")


def _body(nc, tc, d, h_bf16, sim_silu, HDT):
    x_d, swT_d, waT_d, wbT_d = d["x_d"], d["swT_d"], d["waT_d"], d["wbT_d"]
    zpad_d = d["zpad_d"]
    p1_d, p2_d, bnp_d = d["p1_d"], d["p2_d"], d["bnp_d"]
    gm1_d, gb1_d, gm2_d, gb2_d = d["gm1_d"], d["gb1_d"], d["gm2_d"], d["gb2_d"]
    out_d = d["out_d"]

    import contextlib
    ctx = contextlib.ExitStack()
    with ctx:
        wp = ctx.enter_context(tc.tile_pool(name="wp", bufs=1))
        wp2 = ctx.enter_context(tc.tile_pool(name="wp2", bufs=SPC))
        sp = ctx.enter_context(tc.tile_pool(name="sp", bufs=SPC))
        h1p = ctx.enter_context(tc.tile_pool(name="h1p", bufs=SPC))
        h2p = ctx.enter_context(tc.tile_pool(name="h2p", bufs=SPC))
        st = ctx.enter_context(tc.tile_pool(name="st", bufs=SPC))
        st1 = ctx.enter_context(tc.tile_pool(name="st1", bufs=1))
        cps = ctx.enter_context(tc.tile_pool(name="cps", bufs=6, space="PSUM"))
        mps = ctx.enter_context(tc.tile_pool(name="mps", bufs=2, space="PSUM"))
        dram = ctx.enter_context(tc.tile_pool(name="dram", bufs=1, space="DRAM"))
        simp = (ctx.enter_context(tc.tile_pool(name="simp", bufs=1))
                if sim_silu else None)

        # ---- persistent weights / params ----
        swT = wp.tile([C, 9, C], F32R, tag="swT")
        nc.sync.dma_start(out=swT, in_=swT_d)
        gm1 = wp.tile([C, 4], F32, tag="gm1")
        nc.sync.dma_start(out=gm1, in_=gm1_d)
        gb1 = wp.tile([4, C], F32, tag="gb1")
        nc.sync.dma_start(out=gb1, in_=gb1_d)
        gm2 = wp.tile([C, 8], F32, tag="gm2")
        nc.sync.dma_start(out=gm2, in_=gm2_d)
        gb2 = wp.tile([8, C], F32, tag="gb2")
        nc.sync.dma_start(out=gb2, in_=gb2_d)
        bnp = wp.tile([C, 2], F32, tag="bnp")
        nc.sync.dma_start(out=bnp, in_=bnp_d)

        waT, wbT, p1, p2 = [], [], [], []
        for s in range(SPC):
            waT.append(wp2.tile([C, HID], F32R, tag="waT", name=f"waT{s}"))
            nc.sync.dma_start(out=waT[s], in_=waT_d[s])
            wbT.append(wp2.tile([C, 2, C], wbT_d.dtype, tag="wbT", name=f"wbT{s}"))
            nc.sync.dma_start(out=wbT[s], in_=wbT_d[s])
            p1.append(wp2.tile([C, 4], F32, tag="p1", name=f"p1_{s}"))
            nc.sync.dma_start(out=p1[s], in_=p1_d[s])
            p2.append(wp2.tile([C, 4], F32, tag="p2", name=f"p2_{s}"))
            nc.sync.dma_start(out=p2[s], in_=p2_d[s])

        # ---- warm-up collective: absorbs CC ring cold-start early ----
        warm_in = dram.tile([C, 2], F32, tag="warm_in")
        warm_out = dram.tile([C, 2], F32, tag="warm_out", addr_space="Shared")
        nc.sync.dma_start(out=warm_in, in_=bnp_d)
        nc.gpsimd.collective_compute(
            "AllReduce", ALU.add,
            replica_groups=[list(range(NCORES))],
            ins=[warm_in.opt()], outs=[warm_out.opt()],
        )

        # ---- sample-persistent big tiles ----
        s_sb = [sp.tile([C, NPIX], F32, tag="s", name=f"s{i}") for i in range(SPC)]

        # =========== phase 1: conv3x3 + conv1 (uses padded x) ===========
        xpp = ctx.enter_context(tc.tile_pool(name="xpadp", bufs=SPC, side="right"))
        if True:
            xpad = []
            for s in range(SPC):
                xp = xpp.tile([C, PADLEN], F32R, tag="xpad", name=f"xpad{s}")
                xpad.append(xp)
                xr = xp.rearrange("p (r c) -> p r c", c=PADW)
                nc.sync.dma_start(out=xr[:, 0:1, :], in_=zpad_d)
                nc.sync.dma_start(out=xr[:, H + 1:H + 2, :], in_=zpad_d)
                nc.sync.dma_start(out=xr[:, :, 0:1], in_=zpad_d)
                nc.sync.dma_start(out=xr[:, :, PADW - 1:PADW], in_=zpad_d)
                xin = x_d[s].rearrange("p (h w) -> p h w", w=W)
                for q in range(4):
                    r = slice(q * (H // 4), (q + 1) * (H // 4))
                    nc.sync.dma_start(
                        out=xr[:, 1 + q * (H // 4):1 + (q + 1) * (H // 4), 1:W + 1],
                        in_=xin[:, r, :],
                    )

            sstats = [st.tile([C, NTILES, 6], F32, tag="sstats", name=f"sst{i}")
                      for i in range(SPC)]
            h1_sb, k1s, c1s = [], [], []
            for s in range(SPC):
                xr = xpad[s].rearrange("p (r c) -> p r c", c=PADW)
                # conv3x3 for sample s
                for t in range(NTILES):
                    r0 = t * ROWS_PER_TILE
                    ps = cps.tile([C, TILE_N], F32, tag="cps")
                    for tap in range(9):
                        dy, dx = divmod(tap, 3)
                        rhs = xr[:, r0 + dy:r0 + dy + ROWS_PER_TILE, dx:dx + W]
                        nc.tensor.matmul(out=ps, lhsT=swT[:, tap, :], rhs=rhs,
                                         start=(tap == 0), stop=(tap == 8))
                    nc.vector.bn_stats(out=sstats[s][:, t, :], in_=ps)
                    nc.scalar.activation(
                        out=s_sb[s][:, t * TILE_N:(t + 1) * TILE_N], in_=ps,
                        func=AF.Copy)
                # conv1 for sample s (epilogues drain under next sample's conv3x3)
                _conv1(nc, s, xr, cps, st, mps, h1p, waT, p1, gm1, gb1,
                       h1_sb, k1s, c1s, HDT, sim_silu, simp, st1)

            # ---- BN partial from per-sample bn_stats -> AllReduce ----
            bnpart = st1.tile([C, 2], F32, tag="bnpart")
            for s in range(SPC):
                smv = st1.tile([C, 2], F32, tag="smv", name=f"smv{s}")
                nc.vector.bn_aggr(out=smv, in_=sstats[s])
                ssum1 = st1.tile([C, 2], F32, tag=f"ssum1_{s}", name=f"ssum1_{s}")
                # col0: mean*N ; col1: (var+mean^2)*N
                nc.vector.tensor_tensor(out=ssum1[:, 1:2], in0=smv[:, 0:1],
                                        in1=smv[:, 0:1], op=ALU.mult)
                nc.vector.tensor_tensor(out=ssum1[:, 1:2], in0=ssum1[:, 1:2],
                                        in1=smv[:, 1:2], op=ALU.add)
                nc.vector.tensor_copy(out=ssum1[:, 0:1], in_=smv[:, 0:1])
                nc.vector.tensor_scalar(out=ssum1, in0=ssum1,
                                        scalar1=float(NPIX), scalar2=None,
                                        op0=ALU.mult)
                if s == 0:
                    first = ssum1
                else:
                    nc.vector.tensor_tensor(out=bnpart, in0=first, in1=ssum1,
                                            op=ALU.add)
            bn_in = dram.tile([C, 2], F32, tag="bn_in")
            bn_out = dram.tile([C, 2], F32, tag="bn_out", addr_space="Shared")
            nc.sync.dma_start(out=bn_in, in_=bnpart)
            nc.gpsimd.collective_compute(
                "AllReduce", ALU.add,
                replica_groups=[list(range(NCORES))],
                ins=[bn_in.opt()], outs=[bn_out.opt()],
            )
            bnagg = st1.tile([C, 2], F32, tag="bnagg")
            nc.sync.dma_start(out=bnagg, in_=bn_out)

            # BN finalize: k = gamma*rsqrt(var+eps), c = beta - mean*k
            bnm = st1.tile([C, 2], F32, tag="bnm")  # (mean, E2)
            nc.vector.tensor_scalar(out=bnm, in0=bnagg, scalar1=1.0 / BN_COUNT,
                                    scalar2=None, op0=ALU.mult)
            bnv = st1.tile([C, 1], F32, tag="bnv")
            nc.vector.tensor_tensor(out=bnv, in0=bnm[:, 0:1], in1=bnm[:, 0:1],
                                    op=ALU.mult)
            nc.vector.tensor_tensor(out=bnv, in0=bnm[:, 1:2], in1=bnv,
                                    op=ALU.subtract)
            nc.vector.tensor_scalar(out=bnv, in0=bnv, scalar1=EPS, scalar2=None,
                                    op0=ALU.add)
            bnr = _rsqrt(nc, st1, bnv, C, 1, "bnr")
            bn_k = st1.tile([C, 1], F32, tag="bn_k")
            nc.vector.tensor_tensor(out=bn_k, in0=bnr, in1=bnp[:, 0:1], op=ALU.mult)
            bn_c = st1.tile([C, 1], F32, tag="bn_c")
            nc.vector.tensor_tensor(out=bn_c, in0=bnm[:, 0:1], in1=bn_k, op=ALU.mult)
            nc.vector.tensor_tensor(out=bn_c, in0=bnp[:, 1:2], in1=bn_c,
                                    op=ALU.subtract)

        # =========== phase 2: conv2, GN2, BN apply, combine ===========
        if True:
            late = xpp
            h2_sb, css, m2ps = [], [], []
            for s in range(SPC):
                h2 = h2p.tile([C, NPIX], HDT, tag="h2", name=f"h2_{s}")
                h2_sb.append(h2)
                h2st = st.tile([C, NTILES, 6], F32, tag="h2st")
                for t in range(NTILES):
                    ps = cps.tile([C, TILE_N], F32, tag="cps")
                    for half in range(2):
                        nc.tensor.matmul(
                            out=ps, lhsT=wbT[s][:, half, :],
                            rhs=h1_sb[s][:, half, t * TILE_N:(t + 1) * TILE_N],
                            start=(half == 0), stop=(half == 1))
                    nc.vector.bn_stats(out=h2st[:, t, :], in_=ps)
                    dst = h2[:, t * TILE_N:(t + 1) * TILE_N]
                    if s == 0:
                        nc.scalar.activation(out=dst, in_=ps, func=AF.Copy)
                    else:
                        nc.vector.tensor_copy(out=dst, in_=ps)

                # GN2 chain: 8 groups of 16 channels
                mv2 = st.tile([C, 2], F32, tag="gn2mv")
                nc.vector.bn_aggr(out=mv2, in_=h2st)
                rhs_t = st.tile([C, 2], F32, tag="gn2rhs")
                nc.vector.tensor_copy(out=rhs_t[:, 0:1], in_=mv2[:, 0:1])
                nc.vector.tensor_tensor(out=rhs_t[:, 1:2], in0=mv2[:, 0:1],
                                        in1=mv2[:, 0:1], op=ALU.mult)
                nc.vector.tensor_tensor(out=rhs_t[:, 1:2], in0=rhs_t[:, 1:2],
                                        in1=mv2[:, 1:2], op=ALU.add)
                psg = mps.tile([8, 2], F32, tag="mps")
                nc.tensor.matmul(out=psg, lhsT=gm2, rhs=rhs_t, start=True, stop=True)
                gsb = st.tile([8, 2], F32, tag="gn2gsb")
                nc.vector.tensor_copy(out=gsb, in_=psg)
                gv = st.tile([8, 1], F32, tag="gn2gv")
                nc.vector.tensor_tensor(out=gv, in0=gsb[:, 0:1], in1=gsb[:, 0:1],
                                        op=ALU.mult)
                nc.vector.tensor_tensor(out=gv, in0=gsb[:, 1:2], in1=gv,
                                        op=ALU.subtract)
                nc.vector.tensor_scalar(out=gv, in0=gv, scalar1=EPS, scalar2=None,
                                        op0=ALU.add)
                gr = _rsqrt(nc, st, gv, 8, 1, "gn2r")
                rhs2 = st.tile([8, 2], F32, tag="gn2rhs2")
                nc.vector.tensor_copy(out=rhs2[:, 0:1], in_=gsb[:, 0:1])
                nc.vector.tensor_copy(out=rhs2[:, 1:2], in_=gr)
                psb = mps.tile([C, 2], F32, tag="mps")
                nc.tensor.matmul(out=psb, lhsT=gb2, rhs=rhs2, start=True, stop=True)
                mrc = st.tile([C, 2], F32, tag="gn2mrc")  # (m_c, r_c)
                nc.vector.tensor_copy(out=mrc, in_=psb)
                # cs = r*g2*coef ; m2' = m2 - b2c/cs
                cs = st.tile([C, 1], F32, tag="gn2cs")
                nc.vector.tensor_tensor(out=cs, in0=mrc[:, 1:2], in1=p2[s][:, 0:1],
                                        op=ALU.mult)
                nc.vector.tensor_tensor(out=cs, in0=cs, in1=p2[s][:, 2:3],
                                        op=ALU.mult)
                rcs = st.tile([C, 1], F32, tag="gn2rcs")
                nc.vector.reciprocal(out=rcs, in_=cs)
                m2p = st.tile([C, 1], F32, tag="gn2m2p")
                nc.vector.tensor_tensor(out=m2p, in0=p2[s][:, 1:2], in1=rcs,
                                        op=ALU.mult)
                nc.vector.tensor_tensor(out=m2p, in0=mrc[:, 0:1], in1=m2p,
                                        op=ALU.subtract)
                css.append(cs)
                m2ps.append(m2p)

            # ---- BN apply + combine, chunked ----
            for s in range(SPC):
                for ch in range(NCHUNKS):
                    ap = s_sb[s][:, ch * CHUNK:(ch + 1) * CHUNK]
                    _silu_apply(nc, simp, ap, ap, bn_k, bn_c, sim_silu, "bna")
                rpc = CHUNK // W  # rows per chunk
                xr2 = xpad[s].rearrange("p (r c) -> p r c", c=PADW)
                for ch in range(NCHUNKS):
                    sl = slice(ch * CHUNK, (ch + 1) * CHUNK)
                    xint = xr2[:, 1 + ch * rpc:1 + (ch + 1) * rpc, 1:1 + W]
                    outc = late.tile([C, CHUNK], F32, tag="outc")
                    # O3: out = x*pass + silu(bn(s))     (DVE)
                    nc.vector.scalar_tensor_tensor(
                        out=outc, in0=xint.bitcast(F32), scalar=p2[s][:, 3:4],
                        in1=s_sb[s][:, sl], op0=ALU.mult, op1=ALU.add)
                    # O4: h2n = (h2 - m2')*cs             (DVE, bf16 2x)
                    h2n = late.tile([C, CHUNK], HDT, tag="h2n")
                    nc.vector.tensor_scalar(
                        out=h2n, in0=h2_sb[s][:, sl], scalar1=m2ps[s],
                        scalar2=css[s], op0=ALU.subtract, op1=ALU.mult)
                    # O5: out += h2n                      (Pool)
                    nc.gpsimd.tensor_tensor(out=outc, in0=outc, in1=h2n,
                                            op=ALU.add)
                    nc.sync.dma_start(out=out_d[s][:, sl], in_=outc)


# ---------------- host side ----------------

_module_cache = {}


def _get_module(h_bf16=True, sim_silu=False):
    key = (h_bf16, sim_silu)
    if key not in _module_cache:
        _module_cache[key] = build_module(h_bf16=h_bf16, sim_silu=sim_silu)
    return _module_cache[key]


def make_in_maps(x, weights, indices, shared_w, bn_gamma, bn_beta,
                 w1, g1, b1, w2, g2, b2, h_bf16=True):
    """Shard + route on host: returns per-core input dicts."""
    import ml_dtypes
    wb_np = ml_dtypes.bfloat16 if h_bf16 else np.float32

    x = np.asarray(x, np.float32)
    weights = np.asarray(weights, np.float32)
    indices = np.asarray(indices).astype(np.int64)
    shared_w = np.asarray(shared_w, np.float32)
    w1 = np.asarray(w1, np.float32)
    w2 = np.asarray(w2, np.float32)
    g1 = np.asarray(g1, np.float32)
    b1 = np.asarray(b1, np.float32)
    g2 = np.asarray(g2, np.float32)
    b2 = np.asarray(b2, np.float32)

    # shared conv weights -> [C_in, 9, C_out]
    swT = np.ascontiguousarray(shared_w.transpose(1, 2, 3, 0).reshape(C, 9, C))
    bnp = np.stack([np.asarray(bn_gamma, np.float32),
                    np.asarray(bn_beta, np.float32)], axis=1)

    gm1 = np.zeros((C, 4), np.float32)
    for g in range(4):
        gm1[g * 32:(g + 1) * 32, g] = 1.0 / 32.0
    gb1 = np.zeros((4, C), np.float32)
    for g in range(4):
        gb1[g, g * 32:(g + 1) * 32] = 1.0
    gm2 = np.zeros((C, 8), np.float32)
    for g in range(8):
        gm2[g * 16:(g + 1) * 16, g] = 1.0 / 16.0
    gb2 = np.zeros((8, C), np.float32)
    for g in range(8):
        gb2[g, g * 16:(g + 1) * 16] = 1.0

    in_maps = []
    for core in range(NCORES):
        sl = slice(core * SPC, (core + 1) * SPC)
        xs = np.ascontiguousarray(x[sl].reshape(SPC, C, NPIX))
        waT = np.zeros((SPC, C, HID), np.float32)
        wbT = np.zeros((SPC, C, 2, C), np.float32)
        p1 = np.zeros((SPC, C, 4), np.float32)
        p2 = np.zeros((SPC, C, 4), np.float32)
        for s in range(SPC):
            b_idx = core * SPC + s
            idx = int(indices[b_idx])
            e = max(idx - 1, 0)
            coef = float(weights[b_idx]) if idx > 0 else 0.0
            pass_c = float(weights[b_idx]) if idx == 0 else 0.0
            waT[s] = w1[e].T                       # [C, HID]
            w2T = w2[e].T                          # [HID, C]
            wbT[s, :, 0, :] = w2T[:C]
            wbT[s, :, 1, :] = w2T[C:]
            p1[s, :, 0] = g1[e][:C]
            p1[s, :, 1] = g1[e][C:]
            p1[s, :, 2] = b1[e][:C]
            p1[s, :, 3] = b1[e][C:]
            p2[s, :, 0] = g2[e]
            p2[s, :, 1] = b2[e] * coef
            p2[s, :, 2] = max(coef, 1e-30)
            p2[s, :, 3] = pass_c
        in_maps.append(dict(
            x=xs, swT=swT, waT=waT, wbT=wbT.astype(wb_np), p1=p1, p2=p2,
            bnp=bnp, gm1=gm1, gb1=gb1, gm2=gm2, gb2=gb2,
            zpad=np.zeros((C, PADW), np.float32),
        ))
    return in_maps


def kernel(**inputs) -> np.ndarray:
    from concourse import bass_utils

    h_bf16 = os.environ.get("MOE_H_BF16", "1") == "1"
    nc = _get_module(h_bf16=h_bf16, sim_silu=False)
    in_maps = make_in_maps(h_bf16=h_bf16, **inputs)
    res = bass_utils.run_bass_kernel_spmd(
        nc, in_maps, core_ids=list(range(NCORES)),
        trace=os.environ.get("MOE_TRACE", "0") == "1",
    )
    if res.exec_time_ns is not None:
        print(f"HW exec time: {res.exec_time_ns} ns")
    out = np.concatenate([r["out"] for r in res.results], axis=0)
    return out.reshape(B, C, H, W).astype(np.float32)


# revision 15
# speedup vs baseline: 1.6407x; 1.1177x over previous
"""Trainium2 Bass kernel for nn_DecoupledMoEContainer_52596169507001.

Strategy: data-parallel over batch, 2 samples per core across 8 NeuronCores.
Host-side routing: per-sample expert weights are gathered by `indices` so the
device program is uniform (the NEFF does not depend on input values).

Per sample on device:
  shared: s = conv3x3(x)            -> 9 accumulated fp32r matmuls per row-tile
          BN (batch stats)          -> per-channel sum/sumsq, cross-core
                                       AllReduce of [128,2], then SiLU apply
  expert: h1 = conv1x1 (C->HID)     -> 2 fp32r matmuls per row-tile
          GN(8)+SiLU                -> bn_stats + tiny mask-matmuls + rsqrt
          h2 = conv1x1 (HID->C)     -> 2 accumulated matmuls per row-tile
          GN(8)                     -> folded into final combine
  out = silu(bn(s)) + pass*x + (h2 - m2')*cs      (routing coefs from host)
"""

import os

import numpy as np

import concourse.bacc as bacc
import concourse.bass as bass
import concourse.mybir as mybir
import concourse.tile as tile

F32 = mybir.dt.float32
F32R = mybir.dt.float32r
BF16 = mybir.dt.bfloat16
I32 = mybir.dt.int32
AF = mybir.ActivationFunctionType
ALU = mybir.AluOpType

# Problem shapes (hardcoded per spec)
B, C, HID, H, W = 16, 128, 256, 80, 80
NPIX = H * W                     # 6400
NCORES = 8
SPC = B // NCORES                # samples per core = 2
EPS = 1e-5

ROWS_PER_TILE = 5
NTILES = H // ROWS_PER_TILE      # 16
TILE_N = ROWS_PER_TILE * W       # 400
CHUNK = 800
NCHUNKS = NPIX // CHUNK          # 8
PADW = W + 2                     # 82
PADLEN = (H + 2) * PADW          # 6724
BN_COUNT = float(B * NPIX)       # 102400


def _rsqrt(nc, pool, a, p, cols, tag):
    """rstd = 1/sqrt(a) on a [p, cols] fp32 SBUF AP via bit trick + 2 Newton
    iterations (all DVE; ACT Sqrt would force an act-table swap vs Silu)."""
    sh = pool.tile([p, cols], I32, tag=f"{tag}sh")
    nc.vector.tensor_scalar(out=sh, in0=a.bitcast(I32), scalar1=1, scalar2=None,
                            op0=ALU.logical_shift_right)
    yn = pool.tile([p, cols], I32, tag=f"{tag}yn")
    nc.vector.tensor_scalar(out=yn, in0=sh, scalar1=-1, scalar2=None,
                            op0=ALU.bitwise_xor)
    yi = pool.tile([p, cols], I32, tag=f"{tag}yi")
    nc.vector.tensor_scalar(out=yi, in0=yn, scalar1=0x5F3759DF + 1, scalar2=None,
                            op0=ALU.add)
    y = yi.bitcast(F32)
    for it in range(1):
        u = pool.tile([p, cols], F32, tag=f"{tag}u{it}")
        nc.vector.tensor_tensor(out=u, in0=y, in1=y, op=ALU.mult)
        w = pool.tile([p, cols], F32, tag=f"{tag}w{it}")
        nc.vector.tensor_tensor(out=w, in0=u, in1=a, op=ALU.mult)
        z = pool.tile([p, cols], F32, tag=f"{tag}z{it}")
        nc.vector.tensor_scalar(out=z, in0=w, scalar1=-0.5, scalar2=1.5,
                                op0=ALU.mult, op1=ALU.add)
        y2 = pool.tile([p, cols], F32, tag=f"{tag}y{it}")
        nc.vector.tensor_tensor(out=y2, in0=y, in1=z, op=ALU.mult)
        y = y2
    return y


def build_module(h_bf16=True, sim_silu=False):
    nc = bacc.Bacc("TRN2", target_bir_lowering=False, debug=False,
                   num_devices=NCORES)
    HDT = BF16 if h_bf16 else F32
    WBDT = BF16 if h_bf16 else F32R

    x_d = nc.dram_tensor("x", [SPC, C, PADLEN], F32R, kind="ExternalInput").ap()
    swT_d = nc.dram_tensor("swT", [C, 9, C], F32R, kind="ExternalInput").ap()
    waT_d = nc.dram_tensor("waT", [SPC, C, HID], F32R, kind="ExternalInput").ap()
    wbT_d = nc.dram_tensor("wbT", [SPC, C, 2, C], WBDT, kind="ExternalInput").ap()
    p1_d = nc.dram_tensor("p1", [SPC, C, 4], F32, kind="ExternalInput").ap()
    p2_d = nc.dram_tensor("p2", [SPC, C, 4], F32, kind="ExternalInput").ap()
    bnp_d = nc.dram_tensor("bnp", [C, 2], F32, kind="ExternalInput").ap()
    gm1_d = nc.dram_tensor("gm1", [C, 4], F32, kind="ExternalInput").ap()
    gb1_d = nc.dram_tensor("gb1", [4, C], F32, kind="ExternalInput").ap()
    gm2_d = nc.dram_tensor("gm2", [C, 8], F32, kind="ExternalInput").ap()
    gb2_d = nc.dram_tensor("gb2", [8, C], F32, kind="ExternalInput").ap()
    out_d = nc.dram_tensor("out", [SPC, C, NPIX], F32, kind="ExternalOutput").ap()

    with tile.TileContext(nc) as tc:
        _body(nc, tc, locals(), h_bf16=h_bf16, sim_silu=sim_silu, HDT=HDT)
    nc.compile()
    return nc


def _silu_apply(nc, pool, out_ap, in_ap, scale, bias, sim_silu, tag):
    """out = silu(in*scale + bias); CoreSim lacks Silu so emulate there."""
    if not sim_silu:
        nc.scalar.activation(out=out_ap, in_=in_ap, func=AF.Silu,
                             bias=bias, scale=scale)
        return
    p, n = in_ap.shape[0], in_ap.free_size()
    for q in range(2):
        i_ap = in_ap[:, q * (n // 2):(q + 1) * (n // 2)]
        o_ap = out_ap[:, q * (n // 2):(q + 1) * (n // 2)]
        sg = pool.tile([p, n // 2], F32, tag="simsg", name=f"{tag}sg{q}")
        nc.scalar.activation(out=sg, in_=i_ap, func=AF.Sigmoid,
                             bias=bias, scale=scale)
        zz = pool.tile([p, n // 2], F32, tag="simzz", name=f"{tag}zz{q}")
        nc.vector.tensor_scalar(out=zz, in0=i_ap, scalar1=scale, scalar2=bias,
                                op0=ALU.mult, op1=ALU.add)
        nc.vector.tensor_tensor(out=o_ap, in0=sg, in1=zz, op=ALU.mult)



def _conv1(nc, s, xr, cps, st, mps, h1p, waT, p1, gm1, gb1,
           h1_sb, k1s, c1s, HDT, sim_silu, simp, st1):
    """conv1 (C->HID) + GN1 stats/chain + SiLU apply for sample s."""
    h1 = h1p.tile([C, 2, NPIX], HDT, tag="h1", name=f"h1_{s}")
    h1_sb.append(h1)
    h1st = st.tile([C, 2, NTILES, 6], F32, tag="h1st", name=f"h1st{s}")
    for t in range(NTILES):
        r0 = t * ROWS_PER_TILE
        rhs = xr[:, r0 + 1:r0 + 1 + ROWS_PER_TILE, 1:1 + W]
        for half in range(2):
            ps = cps.tile([C, TILE_N], F32, tag="cps", name=f"c1ps{s}_{t}_{half}")
            nc.tensor.matmul(
                out=ps, lhsT=waT[s][:, half * C:(half + 1) * C],
                rhs=rhs, start=True, stop=True)
            nc.vector.bn_stats(out=h1st[:, half, t, :], in_=ps)
            dst = h1[:, half, t * TILE_N:(t + 1) * TILE_N]
            if half == 0:
                nc.scalar.activation(out=dst, in_=ps, func=AF.Copy)
            else:
                nc.vector.tensor_copy(out=dst, in_=ps)

    # GN1 chain: per-half groups of 32 channels
    mv = st.tile([C, 2, 2], F32, tag="gn1mv", name=f"gn1mv{s}")
    for half in range(2):
        nc.vector.bn_aggr(out=mv[:, half, :], in_=h1st[:, half])
    # rhs cols: (m0, q0, m1, q1), q = m*m + var
    rhs_t = st.tile([C, 4], F32, tag="gn1rhs", name=f"gn1rhs{s}")
    mvv = mv.rearrange("p a b -> p (a b)")  # (m0, v0, m1, v1)
    nc.vector.tensor_copy(out=rhs_t[:, 0:4:2], in_=mvv[:, 0:4:2])
    nc.vector.tensor_tensor(out=rhs_t[:, 1:4:2], in0=mvv[:, 0:4:2],
                            in1=mvv[:, 0:4:2], op=ALU.mult)
    nc.vector.tensor_tensor(out=rhs_t[:, 1:4:2], in0=rhs_t[:, 1:4:2],
                            in1=mvv[:, 1:4:2], op=ALU.add)
    psg = mps.tile([4, 4], F32, tag="mps", name=f"gn1psg{s}")
    nc.tensor.matmul(out=psg, lhsT=gm1, rhs=rhs_t, start=True, stop=True)
    gsb = st.tile([4, 4], F32, tag="gn1gsb", name=f"gn1gsb{s}")
    nc.vector.tensor_copy(out=gsb, in_=psg)
    gv = st.tile([4, 2], F32, tag="gn1gv", name=f"gn1gv{s}")
    nc.vector.tensor_tensor(out=gv, in0=gsb[:, 0:4:2], in1=gsb[:, 0:4:2],
                            op=ALU.mult)
    nc.vector.tensor_tensor(out=gv, in0=gsb[:, 1:4:2], in1=gv,
                            op=ALU.subtract)
    nc.vector.tensor_scalar(out=gv, in0=gv, scalar1=EPS, scalar2=None,
                            op0=ALU.add)
    gr = _rsqrt(nc, st, gv, 4, 2, "gn1r")
    rhs2 = st.tile([4, 4], F32, tag="gn1rhs2", name=f"gn1rhs2{s}")
    nc.vector.tensor_copy(out=rhs2[:, 0:4:2], in_=gsb[:, 0:4:2])
    nc.vector.tensor_copy(out=rhs2[:, 1:4:2], in_=gr)
    psb = mps.tile([C, 4], F32, tag="mps", name=f"gn1psb{s}")
    nc.tensor.matmul(out=psb, lhsT=gb1, rhs=rhs2, start=True, stop=True)
    mrc = st.tile([C, 4], F32, tag="gn1mrc", name=f"gn1mrc{s}")
    nc.vector.tensor_copy(out=mrc, in_=psb)
    k1 = st.tile([C, 2], F32, tag="gn1k", name=f"gn1k{s}")
    nc.vector.tensor_tensor(out=k1, in0=mrc[:, 1:4:2], in1=p1[s][:, 0:2],
                            op=ALU.mult)
    c1 = st.tile([C, 2], F32, tag="gn1c", name=f"gn1c{s}")
    nc.vector.tensor_tensor(out=c1, in0=mrc[:, 0:4:2], in1=k1, op=ALU.mult)
    nc.vector.tensor_tensor(out=c1, in0=p1[s][:, 2:4], in1=c1,
                            op=ALU.subtract)
    k1s.append(k1)
    c1s.append(c1)

    # a1 = silu(h1*k + c) in place, per half, chunked
    for half in range(2):
        for ch in range(NCHUNKS):
            ap = h1[:, half, ch * CHUNK:(ch + 1) * CHUNK]
            _silu_apply(nc, simp, ap, ap, k1[:, half:half + 1],
                        c1[:, half:half + 1], sim_silu, "gn1a")


def _body(nc, tc, d, h_bf16, sim_silu, HDT):
    x_d, swT_d, waT_d, wbT_d = d["x_d"], d["swT_d"], d["waT_d"], d["wbT_d"]
    p1_d, p2_d, bnp_d = d["p1_d"], d["p2_d"], d["bnp_d"]
    gm1_d, gb1_d, gm2_d, gb2_d = d["gm1_d"], d["gb1_d"], d["gm2_d"], d["gb2_d"]
    out_d = d["out_d"]

    import contextlib
    ctx = contextlib.ExitStack()
    with ctx:
        wp = ctx.enter_context(tc.tile_pool(name="wp", bufs=1))
        wp2 = ctx.enter_context(tc.tile_pool(name="wp2", bufs=SPC))
        sp = ctx.enter_context(tc.tile_pool(name="sp", bufs=SPC))
        h1p = ctx.enter_context(tc.tile_pool(name="h1p", bufs=SPC))
        h2p = ctx.enter_context(tc.tile_pool(name="h2p", bufs=SPC))
        st = ctx.enter_context(tc.tile_pool(name="st", bufs=SPC))
        st1 = ctx.enter_context(tc.tile_pool(name="st1", bufs=1))
        cps = ctx.enter_context(tc.tile_pool(name="cps", bufs=6, space="PSUM"))
        mps = ctx.enter_context(tc.tile_pool(name="mps", bufs=2, space="PSUM"))
        dram = ctx.enter_context(tc.tile_pool(name="dram", bufs=1, space="DRAM"))
        simp = (ctx.enter_context(tc.tile_pool(name="simp", bufs=1))
                if sim_silu else None)

        # ---- persistent weights / params ----
        swT = wp.tile([C, 9, C], F32R, tag="swT")
        nc.sync.dma_start(out=swT, in_=swT_d)
        gm1 = wp.tile([C, 4], F32, tag="gm1")
        nc.sync.dma_start(out=gm1, in_=gm1_d)
        gb1 = wp.tile([4, C], F32, tag="gb1")
        nc.sync.dma_start(out=gb1, in_=gb1_d)
        gm2 = wp.tile([C, 8], F32, tag="gm2")
        nc.sync.dma_start(out=gm2, in_=gm2_d)
        gb2 = wp.tile([8, C], F32, tag="gb2")
        nc.sync.dma_start(out=gb2, in_=gb2_d)
        bnp = wp.tile([C, 2], F32, tag="bnp")
        nc.sync.dma_start(out=bnp, in_=bnp_d)

        waT, wbT, p1, p2 = [], [], [], []
        for s in range(SPC):
            waT.append(wp2.tile([C, HID], F32R, tag="waT", name=f"waT{s}"))
            nc.sync.dma_start(out=waT[s], in_=waT_d[s])
            wbT.append(wp2.tile([C, 2, C], wbT_d.dtype, tag="wbT", name=f"wbT{s}"))
            nc.sync.dma_start(out=wbT[s], in_=wbT_d[s])
            p1.append(wp2.tile([C, 4], F32, tag="p1", name=f"p1_{s}"))
            nc.sync.dma_start(out=p1[s], in_=p1_d[s])
            p2.append(wp2.tile([C, 4], F32, tag="p2", name=f"p2_{s}"))
            nc.sync.dma_start(out=p2[s], in_=p2_d[s])

        # ---- warm-up collective: absorbs CC ring cold-start early ----
        warm_in = dram.tile([C, 2], F32, tag="warm_in")
        warm_out = dram.tile([C, 2], F32, tag="warm_out", addr_space="Shared")
        nc.sync.dma_start(out=warm_in, in_=bnp_d)
        nc.gpsimd.collective_compute(
            "AllReduce", ALU.add,
            replica_groups=[list(range(NCORES))],
            ins=[warm_in.opt()], outs=[warm_out.opt()],
        )

        # ---- sample-persistent big tiles ----
        s_sb = [sp.tile([C, NPIX], F32, tag="s", name=f"s{i}") for i in range(SPC)]

        # =========== phase 1: conv3x3 + conv1 (uses padded x) ===========
        xpp = ctx.enter_context(tc.tile_pool(name="xpadp", bufs=SPC, side="right"))
        if True:
            xpad = []
            for s in range(SPC):
                xp = xpp.tile([C, PADLEN], F32R, tag="xpad", name=f"xpad{s}")
                xpad.append(xp)
                for q in range(4):
                    sl = slice(q * (PADLEN // 4), (q + 1) * (PADLEN // 4))
                    nc.sync.dma_start(out=xp[:, sl], in_=x_d[s][:, sl])

            sstats = [st.tile([C, NTILES, 6], F32, tag="sstats", name=f"sst{i}")
                      for i in range(SPC)]
            ssum1s = []
            h1_sb, k1s, c1s = [], [], []
            for s in range(SPC):
                xr = xpad[s].rearrange("p (r c) -> p r c", c=PADW)
                # conv3x3 for sample s
                for t in range(NTILES):
                    r0 = t * ROWS_PER_TILE
                    ps = cps.tile([C, TILE_N], F32, tag="cps")
                    for tap in range(9):
                        dy, dx = divmod(tap, 3)
                        rhs = xr[:, r0 + dy:r0 + dy + ROWS_PER_TILE, dx:dx + W]
                        nc.tensor.matmul(out=ps, lhsT=swT[:, tap, :], rhs=rhs,
                                         start=(tap == 0), stop=(tap == 8))
                    nc.vector.bn_stats(out=sstats[s][:, t, :], in_=ps)
                    nc.scalar.activation(
                        out=s_sb[s][:, t * TILE_N:(t + 1) * TILE_N], in_=ps,
                        func=AF.Copy)
                # BN partial for this sample (emitted before conv1 so the
                # AllReduce can trigger as early as possible)
                smv = st1.tile([C, 2], F32, tag=f"smv{s}", name=f"smv{s}")
                nc.vector.bn_aggr(out=smv, in_=sstats[s])
                ssum1 = st1.tile([C, 2], F32, tag=f"ssum1_{s}", name=f"ssum1_{s}")
                # col0: mean*N ; col1: (var+mean^2)*N
                nc.vector.tensor_tensor(out=ssum1[:, 1:2], in0=smv[:, 0:1],
                                        in1=smv[:, 0:1], op=ALU.mult)
                nc.vector.tensor_tensor(out=ssum1[:, 1:2], in0=ssum1[:, 1:2],
                                        in1=smv[:, 1:2], op=ALU.add)
                nc.vector.tensor_copy(out=ssum1[:, 0:1], in_=smv[:, 0:1])
                nc.vector.tensor_scalar(out=ssum1, in0=ssum1,
                                        scalar1=float(NPIX), scalar2=None,
                                        op0=ALU.mult)
                ssum1s.append(ssum1)
                if s == SPC - 1:
                    bnpart = st1.tile([C, 2], F32, tag="bnpart")
                    nc.vector.tensor_tensor(out=bnpart, in0=ssum1s[0],
                                            in1=ssum1s[1], op=ALU.add)
                    bn_in = dram.tile([C, 2], F32, tag="bn_in")
                    bn_out = dram.tile([C, 2], F32, tag="bn_out",
                                       addr_space="Shared")
                    nc.sync.dma_start(out=bn_in, in_=bnpart)
                    nc.gpsimd.collective_compute(
                        "AllReduce", ALU.add,
                        replica_groups=[list(range(NCORES))],
                        ins=[bn_in.opt()], outs=[bn_out.opt()],
                    )
                # conv1 for sample s (epilogues drain under next sample's conv3x3)
                _conv1(nc, s, xr, cps, st, mps, h1p, waT, p1, gm1, gb1,
                       h1_sb, k1s, c1s, HDT, sim_silu, simp, st1)

            _removed = """
            """
            bnagg = st1.tile([C, 2], F32, tag="bnagg")
            nc.sync.dma_start(out=bnagg, in_=bn_out)

            # BN finalize: k = gamma*rsqrt(var+eps), c = beta - mean*k
            bnm = st1.tile([C, 2], F32, tag="bnm")  # (mean, E2)
            nc.vector.tensor_scalar(out=bnm, in0=bnagg, scalar1=1.0 / BN_COUNT,
                                    scalar2=None, op0=ALU.mult)
            bnv = st1.tile([C, 1], F32, tag="bnv")
            nc.vector.tensor_tensor(out=bnv, in0=bnm[:, 0:1], in1=bnm[:, 0:1],
                                    op=ALU.mult)
            nc.vector.tensor_tensor(out=bnv, in0=bnm[:, 1:2], in1=bnv,
                                    op=ALU.subtract)
            nc.vector.tensor_scalar(out=bnv, in0=bnv, scalar1=EPS, scalar2=None,
                                    op0=ALU.add)
            bnr = _rsqrt(nc, st1, bnv, C, 1, "bnr")
            bn_k = st1.tile([C, 1], F32, tag="bn_k")
            nc.vector.tensor_tensor(out=bn_k, in0=bnr, in1=bnp[:, 0:1], op=ALU.mult)
            bn_c = st1.tile([C, 1], F32, tag="bn_c")
            nc.vector.tensor_tensor(out=bn_c, in0=bnm[:, 0:1], in1=bn_k, op=ALU.mult)
            nc.vector.tensor_tensor(out=bn_c, in0=bnp[:, 1:2], in1=bn_c,
                                    op=ALU.subtract)

        # =========== phase 2: conv2, GN2, BN apply, combine ===========
        if True:
            late = xpp
            h2_sb, css, m2ps = [], [], []
            for s in range(SPC):
                h2 = h2p.tile([C, NPIX], HDT, tag="h2", name=f"h2_{s}")
                h2_sb.append(h2)
                h2st = st.tile([C, NTILES, 6], F32, tag="h2st")
                for t in range(NTILES):
                    ps = cps.tile([C, TILE_N], F32, tag="cps")
                    for half in range(2):
                        nc.tensor.matmul(
                            out=ps, lhsT=wbT[s][:, half, :],
                            rhs=h1_sb[s][:, half, t * TILE_N:(t + 1) * TILE_N],
                            start=(half == 0), stop=(half == 1))
                    nc.vector.bn_stats(out=h2st[:, t, :], in_=ps)
                    dst = h2[:, t * TILE_N:(t + 1) * TILE_N]
                    if s == 0:
                        nc.scalar.activation(out=dst, in_=ps, func=AF.Copy)
                    else:
                        nc.vector.tensor_copy(out=dst, in_=ps)

                # GN2 chain: 8 groups of 16 channels
                mv2 = st.tile([C, 2], F32, tag="gn2mv")
                nc.vector.bn_aggr(out=mv2, in_=h2st)
                rhs_t = st.tile([C, 2], F32, tag="gn2rhs")
                nc.vector.tensor_copy(out=rhs_t[:, 0:1], in_=mv2[:, 0:1])
                nc.vector.tensor_tensor(out=rhs_t[:, 1:2], in0=mv2[:, 0:1],
                                        in1=mv2[:, 0:1], op=ALU.mult)
                nc.vector.tensor_tensor(out=rhs_t[:, 1:2], in0=rhs_t[:, 1:2],
                                        in1=mv2[:, 1:2], op=ALU.add)
                psg = mps.tile([8, 2], F32, tag="mps")
                nc.tensor.matmul(out=psg, lhsT=gm2, rhs=rhs_t, start=True, stop=True)
                gsb = st.tile([8, 2], F32, tag="gn2gsb")
                nc.vector.tensor_copy(out=gsb, in_=psg)
                gv = st.tile([8, 1], F32, tag="gn2gv")
                nc.vector.tensor_tensor(out=gv, in0=gsb[:, 0:1], in1=gsb[:, 0:1],
                                        op=ALU.mult)
                nc.vector.tensor_tensor(out=gv, in0=gsb[:, 1:2], in1=gv,
                                        op=ALU.subtract)
                nc.vector.tensor_scalar(out=gv, in0=gv, scalar1=EPS, scalar2=None,
                                        op0=ALU.add)
                gr = _rsqrt(nc, st, gv, 8, 1, "gn2r")
                rhs2 = st.tile([8, 2], F32, tag="gn2rhs2")
                nc.vector.tensor_copy(out=rhs2[:, 0:1], in_=gsb[:, 0:1])
                nc.vector.tensor_copy(out=rhs2[:, 1:2], in_=gr)
                psb = mps.tile([C, 2], F32, tag="mps")
                nc.tensor.matmul(out=psb, lhsT=gb2, rhs=rhs2, start=True, stop=True)
                mrc = st.tile([C, 2], F32, tag="gn2mrc")  # (m_c, r_c)
                nc.vector.tensor_copy(out=mrc, in_=psb)
                # cs = r*g2*coef ; m2' = m2 - b2c/cs
                cs = st.tile([C, 1], F32, tag="gn2cs")
                nc.vector.tensor_tensor(out=cs, in0=mrc[:, 1:2], in1=p2[s][:, 0:1],
                                        op=ALU.mult)
                nc.vector.tensor_tensor(out=cs, in0=cs, in1=p2[s][:, 2:3],
                                        op=ALU.mult)
                rcs = st.tile([C, 1], F32, tag="gn2rcs")
                nc.vector.reciprocal(out=rcs, in_=cs)
                m2p = st.tile([C, 1], F32, tag="gn2m2p")
                nc.vector.tensor_tensor(out=m2p, in0=p2[s][:, 1:2], in1=rcs,
                                        op=ALU.mult)
                nc.vector.tensor_tensor(out=m2p, in0=mrc[:, 0:1], in1=m2p,
                                        op=ALU.subtract)
                css.append(cs)
                m2ps.append(m2p)

            # ---- BN apply + combine, chunked ----
            for s in range(SPC):
                for ch in range(NCHUNKS):
                    ap = s_sb[s][:, ch * CHUNK:(ch + 1) * CHUNK]
                    _silu_apply(nc, simp, ap, ap, bn_k, bn_c, sim_silu, "bna")
                rpc = CHUNK // W  # rows per chunk
                xr2 = xpad[s].rearrange("p (r c) -> p r c", c=PADW)
                for ch in range(NCHUNKS):
                    sl = slice(ch * CHUNK, (ch + 1) * CHUNK)
                    xint = xr2[:, 1 + ch * rpc:1 + (ch + 1) * rpc, 1:1 + W]
                    outc = late.tile([C, CHUNK], F32, tag="outc")
                    # O3: out = x*pass + silu(bn(s))     (DVE)
                    nc.vector.scalar_tensor_tensor(
                        out=outc, in0=xint.bitcast(F32), scalar=p2[s][:, 3:4],
                        in1=s_sb[s][:, sl], op0=ALU.mult, op1=ALU.add)
                    # O4: h2n = (h2 - m2')*cs             (DVE, bf16 2x)
                    h2n = late.tile([C, CHUNK], HDT, tag="h2n")
                    nc.vector.tensor_scalar(
                        out=h2n, in0=h2_sb[s][:, sl], scalar1=m2ps[s],
                        scalar2=css[s], op0=ALU.subtract, op1=ALU.mult)
                    # O5: out += h2n                      (Pool)
                    nc.gpsimd.tensor_tensor(out=outc, in0=outc, in1=h2n,
                                            op=ALU.add)
                    nc.sync.dma_start(out=out_d[s][:, sl], in_=outc)


# ---------------- host side ----------------

_module_cache = {}


def _get_module(h_bf16=True, sim_silu=False):
    key = (h_bf16, sim_silu)
    if key not in _module_cache:
        _module_cache[key] = build_module(h_bf16=h_bf16, sim_silu=sim_silu)
    return _module_cache[key]


def make_in_maps(x, weights, indices, shared_w, bn_gamma, bn_beta,
                 w1, g1, b1, w2, g2, b2, h_bf16=True):
    """Shard + route on host: returns per-core input dicts."""
    import ml_dtypes
    wb_np = ml_dtypes.bfloat16 if h_bf16 else np.float32

    x = np.asarray(x, np.float32)
    weights = np.asarray(weights, np.float32)
    indices = np.asarray(indices).astype(np.int64)
    shared_w = np.asarray(shared_w, np.float32)
    w1 = np.asarray(w1, np.float32)
    w2 = np.asarray(w2, np.float32)
    g1 = np.asarray(g1, np.float32)
    b1 = np.asarray(b1, np.float32)
    g2 = np.asarray(g2, np.float32)
    b2 = np.asarray(b2, np.float32)

    # shared conv weights -> [C_in, 9, C_out]
    swT = np.ascontiguousarray(shared_w.transpose(1, 2, 3, 0).reshape(C, 9, C))
    bnp = np.stack([np.asarray(bn_gamma, np.float32),
                    np.asarray(bn_beta, np.float32)], axis=1)

    gm1 = np.zeros((C, 4), np.float32)
    for g in range(4):
        gm1[g * 32:(g + 1) * 32, g] = 1.0 / 32.0
    gb1 = np.zeros((4, C), np.float32)
    for g in range(4):
        gb1[g, g * 32:(g + 1) * 32] = 1.0
    gm2 = np.zeros((C, 8), np.float32)
    for g in range(8):
        gm2[g * 16:(g + 1) * 16, g] = 1.0 / 16.0
    gb2 = np.zeros((8, C), np.float32)
    for g in range(8):
        gb2[g, g * 16:(g + 1) * 16] = 1.0

    in_maps = []
    for core in range(NCORES):
        sl = slice(core * SPC, (core + 1) * SPC)
        xs = np.pad(x[sl].reshape(SPC, C, H, W), ((0, 0), (0, 0), (1, 1), (1, 1)))
        xs = np.ascontiguousarray(xs.reshape(SPC, C, PADLEN))
        waT = np.zeros((SPC, C, HID), np.float32)
        wbT = np.zeros((SPC, C, 2, C), np.float32)
        p1 = np.zeros((SPC, C, 4), np.float32)
        p2 = np.zeros((SPC, C, 4), np.float32)
        for s in range(SPC):
            b_idx = core * SPC + s
            idx = int(indices[b_idx])
            e = max(idx - 1, 0)
            coef = float(weights[b_idx]) if idx > 0 else 0.0
            pass_c = float(weights[b_idx]) if idx == 0 else 0.0
            waT[s] = w1[e].T                       # [C, HID]
            w2T = w2[e].T                          # [HID, C]
            wbT[s, :, 0, :] = w2T[:C]
            wbT[s, :, 1, :] = w2T[C:]
            p1[s, :, 0] = g1[e][:C]
            p1[s, :, 1] = g1[e][C:]
            p1[s, :, 2] = b1[e][:C]
            p1[s, :, 3] = b1[e][C:]
            p2[s, :, 0] = g2[e]
            p2[s, :, 1] = b2[e] * coef
            p2[s, :, 2] = max(coef, 1e-30)
            p2[s, :, 3] = pass_c
        in_maps.append(dict(
            x=xs, swT=swT, waT=waT, wbT=wbT.astype(wb_np), p1=p1, p2=p2,
            bnp=bnp, gm1=gm1, gb1=gb1, gm2=gm2, gb2=gb2,
        ))
    return in_maps


def kernel(**inputs) -> np.ndarray:
    from concourse import bass_utils

    h_bf16 = os.environ.get("MOE_H_BF16", "1") == "1"
    nc = _get_module(h_bf16=h_bf16, sim_silu=False)
    in_maps = make_in_maps(h_bf16=h_bf16, **inputs)
    res = bass_utils.run_bass_kernel_spmd(
        nc, in_maps, core_ids=list(range(NCORES)),
        trace=os.environ.get("MOE_TRACE", "0") == "1",
    )
    if res.exec_time_ns is not None:
        print(f"HW exec time: {res.exec_time_ns} ns")
    out = np.concatenate([r["out"] for r in res.results], axis=0)
    return out.reshape(B, C, H, W).astype(np.float32)
